# revision 1
# baseline (speedup 1.0000x reference)
"""MoE (top-2, capacity-dropped) Trainium2 kernel — expert-parallel across 8 NeuronCores.

Strategy
--------
Every core receives the FULL token tensor x and replicates the (cheap) routing
computation in fp32; each core owns one expert (its W1/W2/b1/b2 shard arrives as
per-core inputs). Dispatch is a hardware row-gather (dma_gather transpose mode,
bf16) of the <=2048 capacity-kept tokens; the FFN runs in bf16 on the tensor
engine with fp32 PSUM accumulation; the combine is a hardware scatter-add of the
weighted expert outputs into a token-indexed partial buffer, followed by an
8-core ReduceScatter so each core ends up with the final output for its 1/8
token slice (pure data-parallel output sharding -> host just concatenates).

Routing details (matches the reference exactly, in exact arithmetic):
  - top-2 selection on fp32 logits (softmax is monotonic -> argmax of logits);
  - renormalized weights w1 = sigmoid(l1 - l2), w2 = sigmoid(l2 - l1);
  - capacity keeping via a global cumulative sum over tokens per expert
    (rank-0 assignments counted before rank-1), computed with per-tile scans +
    a tile-offset scan (tensor_tensor_scan), all on-device;
  - slot->token map built with a hardware scatter-add into a DRAM staging table
    and gathered back in slot order.
"""

import numpy as np
import ml_dtypes

import concourse.bass as bass
import concourse.tile as tile
from concourse import bacc, mybir
from concourse.bass_utils import run_bass_kernel_spmd
from concourse.masks import make_identity

F32 = mybir.dt.float32
BF16 = mybir.dt.bfloat16
I16 = mybir.dt.int16
I32 = mybir.dt.int32
AF = mybir.ActivationFunctionType
OP = mybir.AluOpType

P = 128
E = 8
TOPK = 2
B, S, D = 2, 4096, 1024
H = 4096
T = B * S                  # 8192 tokens
C = 2048                   # capacity per expert
NT = T // P                # 64 token tiles
DC = D // P                # 8 d-chunks
HC = H // P                # 32 h-chunks
TRASH_SLOT = 2100          # staging rows >= C collect dropped tokens
STAGE_ROWS = 2176          # 17 * 128
PART_ROWS = 8320           # 65 * 128 (8192 tokens + trash rows)
TRASH_TOK = 8200
SLOT_BLOCKS = [(k * 256, 256) for k in range(8)]


def wrap16_const(n):
    """Host-side: slot indices 0..n-1 in the [16, n/16] wrapped layout, tiled to 128 rows."""
    out = np.zeros((16, n // 16), dtype=np.int16)
    j = np.arange(n)
    out[j % 16, j // 16] = j.astype(np.int16)
    return np.tile(out, (8, 1))


def build_moe(debug=False):
    nc = bacc.Bacc("TRN2", target_bir_lowering=False, debug=False, num_devices=E)

    x_in = nc.dram_tensor("x", [T, D], F32, kind="ExternalInput").ap()
    wg_in = nc.dram_tensor("wg", [P, DC, E], F32, kind="ExternalInput").ap()
    sel_in = nc.dram_tensor("sel", [P, E], F32, kind="ExternalInput").ap()
    w1_in = nc.dram_tensor("w1s", [P, DC, H], BF16, kind="ExternalInput").ap()
    w2_in = nc.dram_tensor("w2s", [P, HC, D], BF16, kind="ExternalInput").ap()
    b1_in = nc.dram_tensor("b1s", [P, HC], F32, kind="ExternalInput").ap()
    b2_in = nc.dram_tensor("b2r", [1, D], BF16, kind="ExternalInput").ap()
    gidx_in = nc.dram_tensor("gidx", [P, C // 16], I16, kind="ExternalInput").ap()

    out_sl = nc.dram_tensor("out_slice", [T // E, D], F32, kind="ExternalOutput").ap()

    xbf_dram = nc.dram_tensor("xbf_stage", [T, D], BF16)
    map_stage = nc.dram_tensor("map_stage", [STAGE_ROWS, 64], F32)
    partial = nc.dram_tensor(
        "partial", [PART_ROWS, D], BF16, kind="ExternalOutput" if debug else "Internal"
    )
    rs_out = nc.dram_tensor("rs_out", [T // E, D], BF16)
    if debug:
        dbg_logits = nc.dram_tensor("dbg_logits", [P, NT, E], F32, kind="ExternalOutput").ap()
        dbg_map = nc.dram_tensor("dbg_map", [P, C // P, 64], F32, kind="ExternalOutput").ap()
        dbg_cw = nc.dram_tensor("dbg_cw", [P, NT], F32, kind="ExternalOutput").ap()
        dbg_pos = nc.dram_tensor("dbg_pos", [P, NT], F32, kind="ExternalOutput").ap()

    with tile.TileContext(nc) as tc:
        with (
            tc.tile_pool(name="const", bufs=1) as const,
            tc.tile_pool(name="persist", bufs=1) as persist,
        ):
            # ---------------- constants ----------------
            ident = const.tile([P, P], F32)
            make_identity(nc, ident[:])
            wg_sb = const.tile([P, DC, E], F32)
            nc.sync.dma_start(wg_sb[:], wg_in[:])
            sel_sb = const.tile([P, E], F32)
            nc.sync.dma_start(sel_sb[:], sel_in[:])
            b1_sb = const.tile([P, HC], F32)
            nc.sync.dma_start(b1_sb[:], b1_in[:])
            b2_sb = const.tile([1, D], BF16)
            nc.sync.dma_start(b2_sb[:], b2_in[:])
            ones1 = const.tile([1, P], BF16)
            nc.vector.memset(ones1[:], 1.0)
            gidx_sb = const.tile([P, C // 16], I16)
            nc.sync.dma_start(gidx_sb[:], gidx_in[:])

            # zero the combine partial buffer + map staging table
            with tc.tile_pool(name="zpool", bufs=1) as zpool:
                zero_bf = zpool.tile([P, D], BF16)
                nc.vector.memset(zero_bf[:], 0.0)
                for i in range(PART_ROWS // P):
                    nc.sync.dma_start(partial[i * P:(i + 1) * P, :], zero_bf[:])
                zero_f32 = zpool.tile([P, (STAGE_ROWS // P) * 64], F32)
                nc.vector.memset(zero_f32[:], 0.0)
                nc.sync.dma_start(
                    map_stage[:].rearrange("(a p) c -> p a c", p=P), zero_f32[:].rearrange("p (a c) -> p a c", c=64)
                )

            logits_sb = persist.tile([P, NT, E], F32)

            # ---------------- phase R1: load x, cast to bf16, transpose, logits ----------------
            with (
                tc.tile_pool(name="r1x", bufs=3) as r1x,
                tc.tile_pool(name="r1xb", bufs=3) as r1xb,
                tc.tile_pool(name="r1xt", bufs=3) as r1xt,
                tc.tile_pool(name="r1pst", bufs=2, space="PSUM") as r1pst,
                tc.tile_pool(name="r1psl", bufs=2, space="PSUM") as r1psl,
            ):
                for i in range(NT):
                    x_sb = r1x.tile([P, D], F32)
                    nc.sync.dma_start(x_sb[:], x_in[i * P:(i + 1) * P, :])
                    xb_sb = r1xb.tile([P, D], BF16)
                    nc.gpsimd.tensor_copy(xb_sb[:], x_sb[:])
                    nc.sync.dma_start(xbf_dram[i * P:(i + 1) * P, :], xb_sb[:])

                    lg_ps = r1psl.tile([P, E], F32, space="PSUM")
                    for half in range(2):
                        tr_ps = r1pst.tile([P, 4 * P], F32, space="PSUM")
                        for j in range(4):
                            dc = half * 4 + j
                            nc.tensor.matmul(
                                tr_ps[:, j * P:(j + 1) * P],
                                x_sb[:, dc * P:(dc + 1) * P],
                                ident[:],
                                is_transpose=True,
                                start=(j == 0),
                                stop=(j == 3),
                            )
                        xt_sb = r1xt.tile([P, 4 * P], F32)
                        nc.vector.tensor_copy(xt_sb[:], tr_ps[:])
                        for j in range(4):
                            dc = half * 4 + j
                            nc.tensor.matmul(
                                lg_ps[:],
                                xt_sb[:, j * P:(j + 1) * P],
                                wg_sb[:, dc, :],
                                start=(dc == 0),
                                stop=(dc == DC - 1),
                            )
                    nc.vector.tensor_copy(logits_sb[:, i, :], lg_ps[:])

            # ---------------- phase R2: top-2 + weights (token-tile layout) ----------------
            with (
                tc.tile_pool(name="r2", bufs=1) as r2,
                tc.tile_pool(name="r3ps", bufs=1, space="PSUM") as r3ps,
            ):
                m1 = r2.tile([P, NT], F32)
                nc.vector.tensor_reduce(m1[:], logits_sb[:], axis=mybir.AxisListType.X, op=OP.max)
                oh1 = r2.tile([P, NT, E], F32)
                nc.vector.tensor_tensor(
                    oh1[:], logits_sb[:], m1[:].rearrange("p t -> p t ()").to_broadcast([P, NT, E]),
                    op=OP.is_equal,
                )
                masked = r2.tile([P, NT, E], F32)
                nc.vector.tensor_scalar(masked[:], oh1[:], -1e9, None, op0=OP.mult)
                nc.vector.tensor_tensor(masked[:], masked[:], logits_sb[:], op=OP.add)
                m2 = r2.tile([P, NT], F32)
                nc.vector.tensor_reduce(m2[:], masked[:], axis=mybir.AxisListType.X, op=OP.max)
                oh2 = r2.tile([P, NT, E], F32)
                nc.vector.tensor_tensor(
                    oh2[:], masked[:], m2[:].rearrange("p t -> p t ()").to_broadcast([P, NT, E]),
                    op=OP.is_equal,
                )
                delta = r2.tile([P, NT], F32)
                nc.vector.tensor_tensor(delta[:], m2[:], m1[:], op=OP.subtract)
                w1 = r2.tile([P, NT], F32)
                nc.scalar.activation(w1[:], delta[:], AF.Sigmoid, scale=-1.0)
                w2 = r2.tile([P, NT], F32)
                nc.scalar.activation(w2[:], delta[:], AF.Sigmoid)

                # select this core's expert column: oh_e = sum_E(oh * sel)
                sel_b = sel_sb[:].rearrange("p e -> p () e").to_broadcast([P, NT, E])
                tmp = r2.tile([P, NT, E], F32)
                oh1e = r2.tile([P, NT], F32)
                nc.vector.tensor_tensor(tmp[:], oh1[:], sel_b, op=OP.mult)
                nc.vector.tensor_reduce(oh1e[:], tmp[:], axis=mybir.AxisListType.X, op=OP.max)
                oh2e = r2.tile([P, NT], F32)
                nc.vector.tensor_tensor(tmp[:], oh2[:], sel_b, op=OP.mult)
                nc.vector.tensor_reduce(oh2e[:], tmp[:], axis=mybir.AxisListType.X, op=OP.max)

                # ---------------- phase R3: capacity cumsum in [tile, token] layout ----------
                # transpose oh1e/oh2e [128, 64] -> [64, 128] (packed into one psum bank)
                ohT_ps = r3ps.tile([P, 2 * P], F32, space="PSUM")
                nc.tensor.matmul(ohT_ps[0:NT, 0:P], oh1e[:], ident[:], is_transpose=True, start=True, stop=False)
                nc.tensor.matmul(ohT_ps[0:NT, P:2 * P], oh2e[:], ident[:], is_transpose=True, start=False, stop=True)
                oh_ic = r2.tile([NT, 2, P], F32)
                nc.vector.tensor_copy(oh_ic[:], ohT_ps[0:NT, :].rearrange("a (k p) -> a k p", k=2))

                ic = r2.tile([NT, 2, P], F32)   # per-tile inclusive cumsums, both ranks
                nc.vector.tensor_tensor_scan(
                    ic[:, 0, :], oh_ic[:, 0, :], oh_ic[:, 0, :], 0.0, op0=OP.add, op1=OP.bypass
                )
                nc.vector.tensor_tensor_scan(
                    ic[:, 1, :], oh_ic[:, 1, :], oh_ic[:, 1, :], 0.0, op0=OP.add, op1=OP.bypass
                )
                # tile totals -> [1, 64] via transpose, prefix-scan, back
                sT_ps = r3ps.tile([P, 2 * NT], F32, space="PSUM")
                nc.tensor.matmul(sT_ps[0:1, 0:NT], ic[:, 0, P - 1:P], ident[0:NT, 0:NT], is_transpose=True, start=True, stop=False)
                nc.tensor.matmul(sT_ps[0:1, NT:2 * NT], ic[:, 1, P - 1:P], ident[0:NT, 0:NT], is_transpose=True, start=False, stop=True)
                sT = r2.tile([1, 2, NT], F32)
                nc.vector.tensor_copy(sT[:], sT_ps[0:1, :].rearrange("a (k t) -> a k t", k=2))
                S1 = r2.tile([1, 2, NT], F32)
                nc.vector.tensor_tensor_scan(
                    S1[:, 0, :], sT[:, 0, :], sT[:, 0, :], 0.0, op0=OP.add, op1=OP.bypass
                )
                c0 = r2.tile([1, 1], F32)
                nc.vector.tensor_scalar(c0[:], S1[:, 0, NT - 1:NT], 2048.0, None, op0=OP.min)
                nc.vector.tensor_tensor_scan(
                    S1[:, 1, :], sT[:, 1, :], sT[:, 1, :], c0[:], op0=OP.add, op1=OP.bypass
                )
                offsT = r2.tile([1, 2, NT], F32)
                nc.vector.tensor_tensor(offsT[:], S1[:], sT[:], op=OP.subtract)
                # back-transpose offsets to [64, 1] per rank
                offs = r2.tile([NT, 2, 1], F32)
                for r in range(2):
                    offs_ps = r3ps.tile([P, 1], F32, space="PSUM", name="offs_ps")
                    nc.tensor.matmul(offs_ps[0:NT, :], offsT[:, r, :], ident[0:1, 0:1], is_transpose=True, start=True, stop=True)
                    nc.vector.tensor_copy(offs[:, r, :], offs_ps[0:NT, :])

                cs = r2.tile([NT, 2, P], F32)
                nc.vector.tensor_scalar(cs[:, 0, :], ic[:, 0, :], offs[:, 0, :], None, op0=OP.add)
                nc.vector.tensor_scalar(cs[:, 1, :], ic[:, 1, :], offs[:, 1, :], None, op0=OP.add)

                keep = r2.tile([NT, 2, P], F32)
                nc.vector.tensor_scalar(keep[:], cs[:], float(C), None, op0=OP.is_le)
                k12 = r2.tile([NT, 2, P], F32)
                nc.vector.tensor_tensor(k12[:], keep[:], oh_ic[:], op=OP.mult)

                # pos = k1*cs1 + k2*cs2 + TRASH + (-1 - TRASH)*(k1+k2)
                kcs = r2.tile([NT, 2, P], F32)
                nc.vector.tensor_tensor(kcs[:], k12[:], cs[:], op=OP.mult)
                pos_ic = r2.tile([NT, P], F32)
                nc.vector.tensor_tensor(pos_ic[:], kcs[:, 0, :], kcs[:, 1, :], op=OP.add)
                ksum = r2.tile([NT, P], F32)
                nc.vector.tensor_tensor(ksum[:], k12[:, 0, :], k12[:, 1, :], op=OP.add)
                nc.vector.tensor_scalar(
                    ksum[:], ksum[:], -float(TRASH_SLOT + 1), float(TRASH_SLOT), op0=OP.mult, op1=OP.add
                )
                nc.vector.tensor_tensor(pos_ic[:], pos_ic[:], ksum[:], op=OP.add)

                # back to token layout: pos [128, 64] (int16) and k1/k2 [128, 64]
                pk_ps = r3ps.tile([P, 3 * NT], F32, space="PSUM")
                nc.tensor.matmul(pk_ps[:, 0:NT], pos_ic[:], ident[0:NT, 0:NT], is_transpose=True, start=True, stop=False)
                nc.tensor.matmul(pk_ps[:, NT:2 * NT], k12[:, 0, :], ident[0:NT, 0:NT], is_transpose=True, start=False, stop=False)
                nc.tensor.matmul(pk_ps[:, 2 * NT:3 * NT], k12[:, 1, :], ident[0:NT, 0:NT], is_transpose=True, start=False, stop=True)
                pos_i16 = r2.tile([P, NT], I16)
                nc.vector.tensor_copy(pos_i16[:], pk_ps[:, 0:NT])
                cw_tok = r2.tile([P, NT], F32)
                t1 = r2.tile([P, NT], F32)
                nc.vector.tensor_tensor(cw_tok[:], w1[:], pk_ps[:, NT:2 * NT], op=OP.mult)
                nc.vector.tensor_tensor(t1[:], w2[:], pk_ps[:, 2 * NT:3 * NT], op=OP.mult)
                nc.vector.tensor_tensor(cw_tok[:], cw_tok[:], t1[:], op=OP.add)
                if debug:
                    nc.sync.dma_start(dbg_cw[:], cw_tok[:])
                    pos_f_dbg = r2.tile([P, NT], F32)
                    nc.vector.tensor_copy(pos_f_dbg[:], pk_ps[:, 0:NT])
                    nc.sync.dma_start(dbg_pos[:], pos_f_dbg[:])

                # ---------------- build wrapped-16 idx for the staging scatter -------------
                idx_pos = persist.tile([P, NT, E], I16)   # [128, 512] wrapped: col = tile*8+g
                sh_pos = r2.tile([P, NT], I16)
                mask = [(i + 16) % 32 for i in range(32)]
                nc.vector.stream_shuffle(sh_pos[:], pos_i16[:], mask)
                for g in range(8):
                    q, lower = g // 2, (g % 2 == 0)
                    src = pos_i16 if lower else sh_pos
                    nc.vector.tensor_copy(idx_pos[0:16, :, g], src[q * 32:q * 32 + 16, :])
                for k in range(1, 8):
                    nc.sync.dma_start(idx_pos[16 * k:16 * (k + 1), :, :], idx_pos[0:16, :, :])

                # staging scatter input: rows [token_id+1, cw, 0...]
                stage_f = r2.tile([P, NT, 64], F32)
                nc.vector.memset(stage_f[:], 0.0)
                ids = r2.tile([P, NT], I32)
                nc.gpsimd.iota(ids[:], pattern=[[P, NT]], base=1, channel_multiplier=1)
                nc.vector.tensor_copy(stage_f[:, :, 0], ids[:])
                nc.vector.tensor_copy(stage_f[:, :, 1], cw_tok[:])
                for k4 in range(4):
                    nc.gpsimd.dma_scatter_add(
                        out_ap=map_stage[:],
                        in_ap=stage_f[:, 16 * k4:16 * (k4 + 1), :],
                        idxs_ap=idx_pos[:, 16 * k4:16 * (k4 + 1), :].rearrange("p a b -> p (a b)"),
                        num_idxs=T // 4, num_idxs_reg=T // 4, elem_size=64,
                    )

            # ---------------- slot-order maps ----------------
            with tc.tile_pool(name="mapb", bufs=1) as mapb:
                map_got = persist.tile([P, C // P, 64], F32)
                for k8 in range(8):
                    nc.gpsimd.dma_gather(
                        out_ap=map_got[:, 2 * k8:2 * (k8 + 1), :],
                        in_ap=map_stage[:],
                        idxs_ap=gidx_sb[:, 16 * k8:16 * (k8 + 1)],
                        num_idxs=C // 8, num_idxs_reg=C // 8, elem_size=64,
                    )
                if debug:
                    nc.sync.dma_start(dbg_map[:], map_got[:])
                tok0 = mapb.tile([P, C // P], F32)
                nc.vector.tensor_scalar(tok0[:], map_got[:, :, 0], -1.0, None, op0=OP.add)
                xg_f = mapb.tile([P, C // P], F32)
                nc.vector.tensor_scalar(xg_f[:], tok0[:], 0.0, None, op0=OP.max)
                neg = mapb.tile([P, C // P], F32)
                nc.vector.tensor_scalar(neg[:], tok0[:], 0.0, None, op0=OP.is_lt)
                sc_f = mapb.tile([P, C // P], F32)
                nc.vector.tensor_scalar(sc_f[:], neg[:], float(TRASH_TOK + 1), None, op0=OP.mult)
                nc.vector.tensor_tensor(sc_f[:], sc_f[:], tok0[:], op=OP.add)
                xg_i = mapb.tile([P, C // P], I16)
                nc.vector.tensor_copy(xg_i[:], xg_f[:])
                sc_i = mapb.tile([P, C // P], I16)
                nc.vector.tensor_copy(sc_i[:], sc_f[:])

                idx_xg = persist.tile([P, C // P, E], I16)
                idx_sc = persist.tile([P, C // P, E], I16)
                mask = [(i + 16) % 32 for i in range(32)]
                sh_xg = mapb.tile([P, C // P], I16)
                nc.vector.stream_shuffle(sh_xg[:], xg_i[:], mask)
                sh_sc = mapb.tile([P, C // P], I16)
                nc.vector.stream_shuffle(sh_sc[:], sc_i[:], mask)
                for g in range(8):
                    q, lower = g // 2, (g % 2 == 0)
                    nc.vector.tensor_copy(idx_xg[0:16, :, g], (xg_i if lower else sh_xg)[q * 32:q * 32 + 16, :])
                    nc.vector.tensor_copy(idx_sc[0:16, :, g], (sc_i if lower else sh_sc)[q * 32:q * 32 + 16, :])
                for k in range(1, 8):
                    nc.sync.dma_start(idx_xg[16 * k:16 * (k + 1), :, :], idx_xg[0:16, :, :])
                    nc.sync.dma_start(idx_sc[16 * k:16 * (k + 1), :, :], idx_sc[0:16, :, :])

            # ---------------- weights + dispatch gather ----------------
            wpool_cm = tc.tile_pool(name="wpool", bufs=1)
            wpool = wpool_cm.__enter__()
            w1_sb = wpool.tile([P, DC, H], BF16)
            nc.sync.dma_start(w1_sb[:], w1_in[:])
            w2_sb = wpool.tile([P, HC, D], BF16)
            nc.sync.dma_start(w2_sb[:], w2_in[:])

            xTe_tiles = []
            for k8 in range(8):
                xTe_k = persist.tile([P, DC, 256], BF16, name=f"xTe{k8}")
                nc.gpsimd.dma_gather(
                    out_ap=xTe_k[:],
                    in_ap=xbf_dram[:],
                    idxs_ap=idx_xg[:, 2 * k8:2 * (k8 + 1), :].rearrange("p a b -> p (a b)"),
                    num_idxs=C // 8, num_idxs_reg=C // 8, elem_size=D, transpose=True,
                )
                xTe_tiles.append(xTe_k)

            # ---------------- FFN ----------------
            with (
                tc.tile_pool(name="hT", bufs=2) as hTp,
                tc.tile_pool(name="ypool", bufs=2) as ypool,
                tc.tile_pool(name="hps", bufs=2, space="PSUM") as hps,
                tc.tile_pool(name="yps", bufs=1, space="PSUM") as yps,
            ):
                for (b0, nb) in SLOT_BLOCKS:
                    ntiles = nb // P
                    y_tiles = [
                        [yps.tile([P, 512], F32, space="PSUM", name=f"y{st}{dg}") for dg in range(2)]
                        for st in range(ntiles)
                    ]
                    for hc in range(HC):
                        h_ps = hps.tile([P, nb], F32, space="PSUM", name="hps")
                        for dc in range(DC):
                            nc.tensor.matmul(
                                h_ps[:],
                                w1_sb[:, dc, hc * P:(hc + 1) * P],
                                xTe_tiles[b0 // 256][:, dc, :],
                                start=(dc == 0),
                                stop=(dc == DC - 1),
                            )
                        h_sb = hTp.tile([P, nb], BF16, name="hsb")
                        nc.scalar.activation(h_sb[:], h_ps[:], AF.Gelu_apprx_tanh, bias=b1_sb[:, hc:hc + 1])
                        for st in range(ntiles):
                            for dg in range(2):
                                nc.tensor.matmul(
                                    y_tiles[st][dg][:],
                                    h_sb[:, st * P:(st + 1) * P],
                                    w2_sb[:, hc, dg * 512:(dg + 1) * 512],
                                    start=(hc == 0),
                                    stop=False,
                                )
                    y_sb = ypool.tile([P, ntiles, D], BF16, name="ysb")
                    for st in range(ntiles):
                        tile_idx = b0 // P + st
                        for dg in range(2):
                            nc.tensor.matmul(
                                y_tiles[st][dg][:],
                                ones1[:],
                                b2_sb[:, dg * 512:(dg + 1) * 512],
                                start=False,
                                stop=True,
                            )
                            nc.scalar.activation(
                                y_sb[:, st, dg * 512:(dg + 1) * 512],
                                y_tiles[st][dg][:],
                                AF.Copy,
                                scale=map_got[:, tile_idx, 1:2],
                            )
                    nc.gpsimd.dma_scatter_add(
                        out_ap=partial[:], in_ap=y_sb[:], idxs_ap=idx_sc[:, b0 // P:(b0 + nb) // P, :].rearrange("p a b -> p (a b)"),
                        num_idxs=nb, num_idxs_reg=nb, elem_size=D,
                    )

            wpool_cm.__exit__(None, None, None)

            # ---------------- combine: ReduceScatter + output ----------------
            if debug:
                nc.sync.dma_start(dbg_logits[:], logits_sb[:])
            else:
                nc.gpsimd.collective_compute(
                    "ReduceScatter",
                    OP.add,
                    replica_groups=[list(range(E))],
                    ins=[partial[0:T, :].opt()],
                    outs=[rs_out[:].opt()],
                )
                with tc.tile_pool(name="outp", bufs=2) as outp:
                    for i in range(T // E // P):
                        t_bf = outp.tile([P, D], BF16)
                        nc.sync.dma_start(t_bf[:], rs_out[i * P:(i + 1) * P, :])
                        t_f = outp.tile([P, D], F32)
                        nc.vector.tensor_copy(t_f[:], t_bf[:])
                        nc.sync.dma_start(out_sl[i * P:(i + 1) * P, :], t_f[:])

    nc.compile()
    return nc


_NC_CACHE = {}


def _get_nc():
    if "nc" not in _NC_CACHE:
        _NC_CACHE["nc"] = build_moe()
    return _NC_CACHE["nc"]


def make_inputs(x, Wg, W1, b1, W2, b2):
    """Host-side sharding: per-core input maps."""
    bf = ml_dtypes.bfloat16
    x = np.ascontiguousarray(np.asarray(x, dtype=np.float32).reshape(T, D))
    wg = np.ascontiguousarray(
        np.asarray(Wg, dtype=np.float32).reshape(DC, P, E).transpose(1, 0, 2)
    )
    gidx = wrap16_const(C)
    in_maps = []
    for e in range(E):
        w1s = np.ascontiguousarray(
            np.asarray(W1[e], dtype=np.float32).reshape(DC, P, H).transpose(1, 0, 2).astype(bf)
        )
        w2s = np.ascontiguousarray(
            np.asarray(W2[e], dtype=np.float32).reshape(HC, P, D).transpose(1, 0, 2).astype(bf)
        )
        b1s = np.ascontiguousarray(np.asarray(b1[e], dtype=np.float32).reshape(HC, P).T)
        b2r = np.asarray(b2[e], dtype=np.float32).reshape(1, D).astype(bf)
        sel = np.zeros((P, E), dtype=np.float32)
        sel[:, e] = 1.0
        in_maps.append({
            "x": x, "wg": wg, "sel": sel,
            "w1s": w1s, "w2s": w2s, "b1s": b1s, "b2r": b2r,
            "gidx": gidx,
        })
    return in_maps


def kernel(x, Wg, W1, b1, W2, b2):
    nc = _get_nc()
    in_maps = make_inputs(x, Wg, W1, b1, W2, b2)
    res = run_bass_kernel_spmd(nc, in_maps, list(range(E)))
    out = np.concatenate([res.results[e]["out_slice"] for e in range(E)], axis=0)
    return out.reshape(B, S, D).astype(np.float32)



# revision 3
# speedup vs baseline: 2.3470x; 2.3470x over previous
"""MoE (top-2, capacity-dropped) Trainium2 kernel v2 — expert-parallel, 8 cores.

vs baseline:
- Routing logits sharded per core + AllGather (logits 32KB, xb 2.1MB).
- All-expert capacity cumsum in transposed (tm*8+e, block, token) layout;
  cross-tile offsets via two static-mask matmuls (Msame/Mlow).
- slot->token map built ON-CHIP via one-hot factored matmuls (no DRAM
  scatter/gather staging).
- FFN: weights SBUF-resident, N=512 moving operand, h staged in SBUF bf16,
  W2 per 256-slot half (PSUM: 2 h banks + 4 y banks).
- Combine: y slot-major, AllGathered in 4 chunks overlapped with FFN; home
  cores gather their tokens' <=2 expert rows and combine with weights.
"""

import numpy as np
import ml_dtypes

import concourse.bass as bass
import concourse.tile as tile
from concourse import bacc, mybir
from concourse.bass_utils import run_bass_kernel_spmd
from concourse.masks import make_identity

F32 = mybir.dt.float32
BF16 = mybir.dt.bfloat16
I16 = mybir.dt.int16
AF = mybir.ActivationFunctionType
OP = mybir.AluOpType

P = 128
E = 8
B, S, D = 2, 4096, 1024
H = 4096
T = B * S                  # 8192 tokens
C = 2048                   # capacity per expert
NT = T // P                # 64 token tiles
NTS = NT // E              # 8 token tiles per core slice
TS = T // E                # 1024 tokens per core slice
DC = D // P                # 8 d-chunks
HC = H // P                # 32 h-chunks
NB = 4                     # FFN super-blocks (512 slots each)
SB = C // NB               # 512 slots per super-block
YCH = E * SB               # rows per y AllGather chunk (4096)


def _ge_sum(nc, r2, src, levels, name):
    """acc = sum_k [src >= levels[k]] (compare cascade; all f32)."""
    acc = r2.tile(list(src.shape), F32, name=name)
    tmp = r2.tile(list(src.shape), F32, name=name + "t")
    nc.vector.tensor_scalar(acc[:], src[:], float(levels[0]), None, op0=OP.is_ge)
    for lv in levels[1:]:
        nc.vector.tensor_scalar(tmp[:], src[:], float(lv), None, op0=OP.is_ge)
        nc.vector.tensor_tensor(acc[:], acc[:], tmp[:], op=OP.add)
    return acc


def build_moe(debug=False):
    nc = bacc.Bacc("TRN2", target_bir_lowering=False, debug=False, num_devices=E)

    xs_in = nc.dram_tensor("xs", [TS, D], F32, kind="ExternalInput").ap()
    wg_in = nc.dram_tensor("wg", [P, DC, E], F32, kind="ExternalInput").ap()
    w1_in = nc.dram_tensor("w1s", [P, HC, DC, P], BF16, kind="ExternalInput").ap()
    w2_in = nc.dram_tensor("w2s", [P, HC, D], BF16, kind="ExternalInput").ap()
    b1_in = nc.dram_tensor("b1s", [P, HC], F32, kind="ExternalInput").ap()
    b2_in = nc.dram_tensor("b2r", [1, D], BF16, kind="ExternalInput").ap()
    esel_in = nc.dram_tensor("esel", [P, E], F32, kind="ExternalInput").ap()
    hm64_in = nc.dram_tensor("hm64", [P, NT], F32, kind="ExternalInput").ap()
    erow_in = nc.dram_tensor("erow", [P, E], F32, kind="ExternalInput").ap()
    msame_in = nc.dram_tensor("msame", [P, P], F32, kind="ExternalInput").ap()
    mlow_in = nc.dram_tensor("mlow", [P, P], F32, kind="ExternalInput").ap()
    lo_in = nc.dram_tensor("loall", [P, NT, P], BF16, kind="ExternalInput").ap()
    hi_in = nc.dram_tensor("hiall", [P, NT, 16], F32, kind="ExternalInput").ap()
    thi_in = nc.dram_tensor("thi", [P, NT], F32, kind="ExternalInput").ap()
    tlo_in = nc.dram_tensor("tlo", [P, 1], F32, kind="ExternalInput").ap()

    out_sl = nc.dram_tensor("out_slice", [TS, D], F32, kind="ExternalOutput").ap()

    xb_sl_dram = nc.dram_tensor("xb_slice", [TS, D], BF16)
    xb_all = nc.dram_tensor("xb_all", [T, D], BF16, addr_space="Shared")
    lg_sl_dram = nc.dram_tensor("lg_slice", [TS, E], F32)
    lg_all = nc.dram_tensor("lg_all", [T, E], F32, addr_space="Shared")
    yag_in = nc.dram_tensor("yag_in", [C, D], BF16)
    yag_out = nc.dram_tensor("yag_out", [NB * YCH, D], BF16, addr_space="Shared")

    if debug:
        dbg_lg = nc.dram_tensor("dbg_lg", [P, NT, E], F32, kind="ExternalOutput").ap()
        dbg_pos1 = nc.dram_tensor("dbg_pos1", [P, NT], F32, kind="ExternalOutput").ap()
        dbg_pos2 = nc.dram_tensor("dbg_pos2", [P, NT], F32, kind="ExternalOutput").ap()
        dbg_map = nc.dram_tensor("dbg_map", [P, 16], F32, kind="ExternalOutput").ap()
        dbg_gs1 = nc.dram_tensor("dbg_gs1", [P, NT], F32, kind="ExternalOutput").ap()
        dbg_gs2 = nc.dram_tensor("dbg_gs2", [P, NT], F32, kind="ExternalOutput").ap()
        dbg_cw1 = nc.dram_tensor("dbg_cw1", [P, NT], F32, kind="ExternalOutput").ap()
        dbg_cw2 = nc.dram_tensor("dbg_cw2", [P, NT], F32, kind="ExternalOutput").ap()

    with tile.TileContext(nc) as tc:
        with (
            tc.tile_pool(name="const", bufs=1) as const,
            tc.tile_pool(name="persist", bufs=1) as persist,
            tc.tile_pool(name="w1pool", bufs=1) as w1pool,
        ):
            # ---------------- constants ----------------
            ident = const.tile([P, P], F32)
            make_identity(nc, ident[:])
            wg_sb = const.tile([P, DC, E], F32)
            nc.sync.dma_start(wg_sb[:], wg_in[:])
            b1_sb = const.tile([P, HC], F32)
            nc.sync.dma_start(b1_sb[:], b1_in[:])
            b2_sb = const.tile([1, D], BF16)
            nc.sync.dma_start(b2_sb[:], b2_in[:])
            ones1 = const.tile([1, P], BF16)
            nc.vector.memset(ones1[:], 1.0)

            # W1 resident (hc-major layout) — stream per hc chunk
            w1_sb = w1pool.tile([P, HC, DC, P], BF16)
            for hc in range(HC):
                nc.sync.dma_start(w1_sb[:, hc, :, :], w1_in[:, hc, :, :])

            # ---------------- P1: slice logits + bf16 cast ----------------
            with (
                tc.tile_pool(name="p1x", bufs=3) as p1x,
                tc.tile_pool(name="p1xb", bufs=3) as p1xb,
                tc.tile_pool(name="p1xt", bufs=3) as p1xt,
                tc.tile_pool(name="p1lg", bufs=2) as p1lg,
                tc.tile_pool(name="p1pst", bufs=2, space="PSUM") as p1pst,
                tc.tile_pool(name="p1psl", bufs=2, space="PSUM") as p1psl,
            ):
                for i in range(NTS):
                    x_sb = p1x.tile([P, D], F32)
                    nc.sync.dma_start(x_sb[:], xs_in[i * P:(i + 1) * P, :])
                    xb_sb = p1xb.tile([P, D], BF16)
                    nc.vector.tensor_copy(xb_sb[:], x_sb[:])
                    nc.sync.dma_start(xb_sl_dram[i * P:(i + 1) * P, :], xb_sb[:])

                    lg_ps = p1psl.tile([P, E], F32, space="PSUM")
                    for half in range(2):
                        tr_ps = p1pst.tile([P, 4 * P], F32, space="PSUM")
                        for j in range(4):
                            dc = half * 4 + j
                            nc.tensor.matmul(
                                tr_ps[:, j * P:(j + 1) * P],
                                x_sb[:, dc * P:(dc + 1) * P],
                                ident[:],
                                is_transpose=True,
                                start=(j == 0),
                                stop=(j == 3),
                            )
                        xt_sb = p1xt.tile([P, 4 * P], F32)
                        nc.vector.tensor_copy(xt_sb[:], tr_ps[:])
                        for j in range(4):
                            dc = half * 4 + j
                            nc.tensor.matmul(
                                lg_ps[:],
                                xt_sb[:, j * P:(j + 1) * P],
                                wg_sb[:, dc, :],
                                start=(dc == 0),
                                stop=(dc == DC - 1),
                            )
                    lg_sb = p1lg.tile([P, E], F32)
                    nc.vector.tensor_copy(lg_sb[:], lg_ps[:])
                    nc.sync.dma_start(lg_sl_dram[i * P:(i + 1) * P, :], lg_sb[:])

            # AllGather logits first (routing critical path), then xb
            nc.gpsimd.collective_compute(
                "AllGather", OP.bypass, replica_groups=[list(range(E))],
                ins=[lg_sl_dram[:].opt()], outs=[lg_all[:].opt()],
            )
            nc.gpsimd.collective_compute(
                "AllGather", OP.bypass, replica_groups=[list(range(E))],
                ins=[xb_sl_dram[:].opt()], outs=[xb_all[:].opt()],
            )

            # persist tiles used across phases
            cw1_my = persist.tile([P, E], F32)
            cw2_my = persist.tile([P, E], F32)
            idx_h1 = persist.tile([P, TS // 16], I16)   # home gather idx (wrapped)
            idx_h2 = persist.tile([P, TS // 16], I16)
            idx_x = persist.tile([P, C // 16], I16)     # dispatch gather idx

            # ---------------- P2: routing (replicated, from lg_all) ------
            with (
                tc.tile_pool(name="r2", bufs=1) as r2,
                tc.tile_pool(name="ohps", bufs=2, space="PSUM") as ohps,
                tc.tile_pool(name="mmps", bufs=1, space="PSUM") as mmps,
                tc.tile_pool(name="bkps", bufs=1, space="PSUM") as bkps,
                tc.tile_pool(name="mapps", bufs=1, space="PSUM") as mapps,
            ):
                esel_sb = r2.tile([P, E], F32)
                nc.sync.dma_start(esel_sb[:], esel_in[:])
                hm64_sb = r2.tile([P, NT], F32)
                nc.sync.dma_start(hm64_sb[:], hm64_in[:])
                erow_sb = r2.tile([P, E], F32)
                nc.sync.dma_start(erow_sb[:], erow_in[:])
                msame_sb = r2.tile([P, P], F32)
                nc.sync.dma_start(msame_sb[:], msame_in[:])
                mlow_sb = r2.tile([P, P], F32)
                nc.sync.dma_start(mlow_sb[:], mlow_in[:])
                lo_sb = r2.tile([P, NT, P], BF16)
                nc.sync.dma_start(lo_sb[:], lo_in[:])
                hi_sb = r2.tile([P, NT, 16], F32)
                nc.sync.dma_start(hi_sb[:], hi_in[:])
                thi_sb = r2.tile([P, NT], F32)
                nc.sync.dma_start(thi_sb[:], thi_in[:])
                tlo_sb = r2.tile([P, 1], F32)
                nc.sync.dma_start(tlo_sb[:], tlo_in[:])

                lgt = r2.tile([P, NT, E], F32)
                nc.sync.dma_start(
                    lgt[:], lg_all[:].rearrange("(a p) e -> p a e", p=P)
                )
                if debug:
                    nc.sync.dma_start(dbg_lg[:], lgt[:])
                # ---- top-2 (token-major) ----
                m1 = r2.tile([P, NT], F32)
                nc.vector.tensor_reduce(m1[:], lgt[:], axis=mybir.AxisListType.X, op=OP.max)
                oh1 = r2.tile([P, NT, E], F32)
                nc.vector.tensor_tensor(
                    oh1[:], lgt[:], m1[:].rearrange("p t -> p t ()").to_broadcast([P, NT, E]),
                    op=OP.is_equal,
                )
                masked = r2.tile([P, NT, E], F32)
                nc.vector.tensor_scalar(masked[:], oh1[:], -1e9, None, op0=OP.mult)
                nc.vector.tensor_tensor(masked[:], masked[:], lgt[:], op=OP.add)
                m2 = r2.tile([P, NT], F32)
                nc.vector.tensor_reduce(m2[:], masked[:], axis=mybir.AxisListType.X, op=OP.max)
                oh2 = r2.tile([P, NT, E], F32)
                nc.vector.tensor_tensor(
                    oh2[:], masked[:], m2[:].rearrange("p t -> p t ()").to_broadcast([P, NT, E]),
                    op=OP.is_equal,
                )
                delta = r2.tile([P, NT], F32)
                nc.vector.tensor_tensor(delta[:], m2[:], m1[:], op=OP.subtract)
                wr1 = r2.tile([P, NT], F32)
                nc.scalar.activation(wr1[:], delta[:], AF.Sigmoid, scale=-1.0)
                wr2 = r2.tile([P, NT], F32)
                nc.scalar.activation(wr2[:], delta[:], AF.Sigmoid)

                # ---- all-expert capacity cumsum, P2 layout (tm*8+e, b, tok) ----
                ohs = [oh1, oh2]
                csm1T = []      # token-major (cs-1)*keep*oh per rank  [P, NT, E]
                kT = []         # token-major keep*oh per rank         [P, NT, E]
                base1 = None
                for r in range(2):
                    ohT_ps = ohps.tile([P, 4 * P], F32, space="PSUM", name="ohT")
                    ohsv = ohs[r][:].rearrange("p a e -> p (a e)")
                    for b in range(4):
                        nc.tensor.matmul(
                            ohT_ps[:, b * P:(b + 1) * P],
                            ohsv[:, b * P:(b + 1) * P],
                            ident[:],
                            is_transpose=True,
                            start=(b == 0), stop=(b == 3),
                        )
                    ohT = r2.tile([P, 4, P], F32, name=f"ohTs{r}")
                    nc.vector.tensor_copy(ohT[:], ohT_ps[:].rearrange("p (b t) -> p b t", b=4))
                    ic = r2.tile([P, 4, P], F32, name=f"ic{r}")
                    for b in range(4):
                        nc.vector.tensor_tensor_scan(
                            ic[:, b, :], ohT[:, b, :], ohT[:, b, :], 0.0,
                            op0=OP.add, op1=OP.bypass,
                        )
                    cnt = r2.tile([P, 4], F32, name=f"cnt{r}")
                    nc.vector.tensor_copy(cnt[:], ic[:, :, P - 1])
                    # cross-tile offsets: same-expert block totals + intra lower
                    mm_ps = mmps.tile([P, 8], F32, space="PSUM", name="mm")
                    nc.tensor.matmul(mm_ps[:, 0:4], msame_sb[:], cnt[:], start=True, stop=False)
                    nc.tensor.matmul(mm_ps[:, 4:8], mlow_sb[:], cnt[:], start=False, stop=True)
                    mm_sb = r2.tile([P, 8], F32, name=f"mmsb{r}")
                    nc.vector.tensor_copy(mm_sb[:], mm_ps[:])
                    btot_i = r2.tile([P, 4], F32, name=f"bti{r}")
                    nc.vector.tensor_tensor_scan(
                        btot_i[:], mm_sb[:, 0:4], mm_sb[:, 0:4], 0.0,
                        op0=OP.add, op1=OP.bypass,
                    )
                    offs = r2.tile([P, 4], F32, name=f"offs{r}")
                    nc.vector.tensor_tensor(offs[:], btot_i[:], mm_sb[:, 0:4], op=OP.subtract)
                    nc.vector.tensor_tensor(offs[:], offs[:], mm_sb[:, 4:8], op=OP.add)
                    if r == 1:
                        nc.vector.tensor_scalar(offs[:], offs[:], base1[:], None, op0=OP.add)
                    cs = r2.tile([P, 4, P], F32, name=f"cs{r}")
                    for b in range(4):
                        nc.vector.tensor_scalar(
                            cs[:, b, :], ic[:, b, :], offs[:, b:b + 1], None, op0=OP.add
                        )
                    if r == 0:
                        # rank-1 base: min(total rank-0 assigned per expert, C);
                        # mm_sb[:, 0:4] holds per-expert block totals (Msame).
                        n0 = r2.tile([P, 1], F32)
                        nc.vector.tensor_reduce(n0[:], mm_sb[:, 0:4], axis=mybir.AxisListType.X, op=OP.add)
                        base1 = r2.tile([P, 1], F32)
                        nc.vector.tensor_scalar(base1[:], n0[:], float(C), None, op0=OP.min)
                    keep = r2.tile([P, 4, P], F32, name=f"keep{r}")
                    nc.vector.tensor_scalar(keep[:], cs[:], float(C), None, op0=OP.is_le)
                    kk = r2.tile([P, 4, P], F32, name=f"kk{r}")
                    nc.vector.tensor_tensor(kk[:], keep[:], ohT[:], op=OP.mult)
                    ksl = r2.tile([P, 4, P], F32, name=f"ksl{r}")
                    nc.vector.tensor_scalar(ksl[:], cs[:], -1.0, None, op0=OP.add)
                    nc.vector.tensor_tensor(ksl[:], ksl[:], kk[:], op=OP.mult)
                    # transpose back to token-major (two 1-bank psum tiles)
                    bk1 = bkps.tile([P, 4 * P], F32, space="PSUM", name="bk1")
                    bk2 = bkps.tile([P, 4 * P], F32, space="PSUM", name="bk2")
                    for b in range(4):
                        nc.tensor.matmul(
                            bk1[:, b * P:(b + 1) * P], ksl[:, b, :], ident[:],
                            is_transpose=True, start=(b == 0), stop=(b == 3),
                        )
                    for b in range(4):
                        nc.tensor.matmul(
                            bk2[:, b * P:(b + 1) * P], kk[:, b, :], ident[:],
                            is_transpose=True, start=(b == 0), stop=(b == 3),
                        )
                    cT = r2.tile([P, NT, E], F32, name=f"cT{r}")
                    nc.vector.tensor_copy(cT[:], bk1[:].rearrange("p (a e) -> p a e", e=E))
                    kTr = r2.tile([P, NT, E], F32, name=f"kTr{r}")
                    nc.vector.tensor_copy(kTr[:], bk2[:].rearrange("p (a e) -> p a e", e=E))
                    csm1T.append(cT)
                    kT.append(kTr)

                # ---- home-side indices (token-major, all tiles) ----
                esel_b = esel_sb[:].rearrange("p e -> p () e").to_broadcast([P, NT, E])
                erow_b = erow_sb[:].rearrange("p e -> p () e").to_broadcast([P, NT, E])
                tmp3 = r2.tile([P, NT, E], F32)
                gs = []
                cwf = []
                for r in range(2):
                    pos = r2.tile([P, NT], F32, name=f"pos{r}")
                    nc.vector.tensor_reduce(pos[:], csm1T[r][:], axis=mybir.AxisListType.X, op=OP.add)
                    keep_s = r2.tile([P, NT], F32, name=f"ks{r}")
                    nc.vector.tensor_reduce(keep_s[:], kT[r][:], axis=mybir.AxisListType.X, op=OP.max)
                    nc.vector.tensor_tensor(tmp3[:], ohs[r][:], erow_b, op=OP.mult)
                    es = r2.tile([P, NT], F32, name=f"es{r}")
                    nc.vector.tensor_reduce(es[:], tmp3[:], axis=mybir.AxisListType.X, op=OP.max)
                    # chunked-AG row index: 4096*(s//512) + 512*e + s%512
                    q = _ge_sum(nc, r2, pos, [512.0, 1024.0, 1536.0], f"q{r}")
                    g = r2.tile([P, NT], F32, name=f"g{r}")
                    rem = r2.tile([P, NT], F32, name=f"rm{r}")
                    nc.vector.tensor_scalar(rem[:], q[:], -512.0, None, op0=OP.mult)
                    nc.vector.tensor_tensor(rem[:], rem[:], pos[:], op=OP.add)
                    nc.vector.tensor_scalar(g[:], q[:], 4096.0, None, op0=OP.mult)
                    t2 = r2.tile([P, NT], F32, name=f"t2{r}")
                    nc.vector.tensor_scalar(t2[:], es[:], 512.0, None, op0=OP.mult)
                    nc.vector.tensor_tensor(g[:], g[:], t2[:], op=OP.add)
                    nc.vector.tensor_tensor(g[:], g[:], rem[:], op=OP.add)
                    cw = r2.tile([P, NT], F32, name=f"cw{r}")
                    wsrc = wr1 if r == 0 else wr2
                    nc.vector.tensor_tensor(cw[:], wsrc[:], keep_s[:], op=OP.mult)
                    gs.append(g)
                    cwf.append(cw)
                    if debug:
                        nc.sync.dma_start([dbg_gs1, dbg_gs2][r][:], g[:])
                        nc.sync.dma_start([dbg_cw1, dbg_cw2][r][:], cw[:])
                        nc.sync.dma_start([dbg_pos1, dbg_pos2][r][:], pos[:])

                # select MY home block (hm64 mask + log-fold), build wrapped idx
                for r in range(2):
                    msk_g = r2.tile([P, NT], F32, name=f"mg{r}")
                    nc.vector.tensor_tensor(msk_g[:], gs[r][:], hm64_sb[:], op=OP.mult)
                    msk_c = r2.tile([P, NT], F32, name=f"mc{r}")
                    nc.vector.tensor_tensor(msk_c[:], cwf[r][:], hm64_sb[:], op=OP.mult)
                    for half in (32, 16, 8):
                        nc.vector.tensor_tensor(
                            msk_g[:, 0:half], msk_g[:, 0:half], msk_g[:, half:2 * half], op=OP.add
                        )
                        nc.vector.tensor_tensor(
                            msk_c[:, 0:half], msk_c[:, 0:half], msk_c[:, half:2 * half], op=OP.add
                        )
                    nc.vector.tensor_copy([cw1_my, cw2_my][r][:], msk_c[:, 0:E])
                    gi = r2.tile([P, E], I16, name=f"gi{r}")
                    nc.vector.tensor_copy(gi[:], msk_g[:, 0:E])
                    sh16 = [(i + 16) % 32 for i in range(32)]
                    gish = r2.tile([P, E], I16, name=f"gish{r}")
                    nc.vector.stream_shuffle(gish[:], gi[:], sh16)
                    idxh = [idx_h1, idx_h2][r]
                    idxh_v = idxh[0:16, :].rearrange("r (c q) -> r c q", q=8)
                    for qq in range(8):
                        src = gi if qq % 2 == 0 else gish
                        nc.vector.tensor_copy(
                            idxh_v[:, :, qq],
                            src[(qq // 2) * 32:(qq // 2) * 32 + 16, :],
                        )
                    for k in range(1, 8):
                        nc.sync.dma_start(idxh[16 * k:16 * (k + 1), :], idxh[0:16, :])

                # ---- expert-side slot->token map (my expert) ----
                ksl_e = r2.tile([P, NT], F32)
                k_e = r2.tile([P, NT], F32)
                acc = r2.tile([P, NT], F32)
                for r in range(2):
                    nc.vector.tensor_tensor(tmp3[:], csm1T[r][:], esel_b, op=OP.mult)
                    nc.vector.tensor_reduce(
                        (acc if r else ksl_e)[:], tmp3[:], axis=mybir.AxisListType.X, op=OP.add
                    )
                    if r:
                        nc.vector.tensor_tensor(ksl_e[:], ksl_e[:], acc[:], op=OP.add)
                    nc.vector.tensor_tensor(tmp3[:], kT[r][:], esel_b, op=OP.mult)
                    nc.vector.tensor_reduce(
                        (acc if r else k_e)[:], tmp3[:], axis=mybir.AxisListType.X, op=OP.max
                    )
                    if r:
                        nc.vector.tensor_tensor(k_e[:], k_e[:], acc[:], op=OP.max)
                # chi = s//128 in [0,16), remc = s%128 via two-level cascade
                q8 = _ge_sum(nc, r2, ksl_e, [512.0, 1024.0, 1536.0], "q8")
                s1 = r2.tile([P, NT], F32)
                nc.vector.tensor_scalar(s1[:], q8[:], -512.0, None, op0=OP.mult)
                nc.vector.tensor_tensor(s1[:], s1[:], ksl_e[:], op=OP.add)
                c3 = _ge_sum(nc, r2, s1, [128.0, 256.0, 384.0], "c3")
                chi = r2.tile([P, NT], F32)
                nc.vector.tensor_scalar(chi[:], q8[:], 4.0, None, op0=OP.mult)
                nc.vector.tensor_tensor(chi[:], chi[:], c3[:], op=OP.add)
                remc = r2.tile([P, NT], F32)
                nc.vector.tensor_scalar(remc[:], c3[:], -128.0, None, op0=OP.mult)
                nc.vector.tensor_tensor(remc[:], remc[:], s1[:], op=OP.add)
                ktlo = r2.tile([P, NT], F32)
                nc.vector.tensor_scalar(ktlo[:], k_e[:], tlo_sb[:], None, op0=OP.mult)
                kthi = r2.tile([P, NT], F32)
                nc.vector.tensor_tensor(kthi[:], k_e[:], thi_sb[:], op=OP.mult)

                o_all = r2.tile([P, NT, P], BF16)   # [s%128 == lo]
                nc.vector.tensor_tensor(
                    o_all[:], lo_sb[:],
                    remc[:].rearrange("p t -> p t ()").to_broadcast([P, NT, P]),
                    op=OP.is_equal,
                )
                v0 = r2.tile([P, NT, 16], F32)
                nc.vector.tensor_tensor(
                    v0[:], hi_sb[:],
                    chi[:].rearrange("p t -> p t ()").to_broadcast([P, NT, 16]),
                    op=OP.is_equal,
                )
                v_all = r2.tile([P, NT, 2, 16], BF16)
                nc.vector.tensor_tensor(
                    v_all[:, :, 0, :], v0[:],
                    ktlo[:].rearrange("p t -> p t ()").to_broadcast([P, NT, 16]),
                    op=OP.mult,
                )
                nc.vector.tensor_tensor(
                    v_all[:, :, 1, :], v0[:],
                    kthi[:].rearrange("p t -> p t ()").to_broadcast([P, NT, 16]),
                    op=OP.mult,
                )
                map_ps = mapps.tile([P, 2, 16], F32, space="PSUM")
                for j in range(NT):
                    nc.tensor.matmul(
                        map_ps[:].rearrange("p a b -> p (a b)"),
                        o_all[:, j, :],
                        v_all[:, j, :, :].rearrange("p a b -> p (a b)"),
                        start=(j == 0), stop=(j == NT - 1),
                    )
                map_sb = r2.tile([P, 2, 16], F32)
                nc.vector.tensor_copy(map_sb[:], map_ps[:])
                map_tok = r2.tile([P, 16], F32)
                nc.vector.tensor_scalar(map_tok[:], map_sb[:, 1, :], 64.0, None, op0=OP.mult)
                nc.vector.tensor_tensor(map_tok[:], map_tok[:], map_sb[:, 0, :], op=OP.add)
                if debug:
                    nc.sync.dma_start(dbg_map[:], map_tok[:])
                mi = r2.tile([P, 16], I16)
                nc.vector.tensor_copy(mi[:], map_tok[:])
                sh16 = [(i + 16) % 32 for i in range(32)]
                mish = r2.tile([P, 16], I16)
                nc.vector.stream_shuffle(mish[:], mi[:], sh16)
                idxx_v = idx_x[0:16, :].rearrange("r (h q) -> r h q", q=8)
                for qq in range(8):
                    src = mi if qq % 2 == 0 else mish
                    nc.vector.tensor_copy(
                        idxx_v[:, :, qq],
                        src[(qq // 2) * 32:(qq // 2) * 32 + 16, :],
                    )
                for k in range(1, 8):
                    nc.sync.dma_start(idx_x[16 * k:16 * (k + 1), :], idx_x[0:16, :])

            # ---------------- P3: FFN ----------------
            with (
                tc.tile_pool(name="w2pool", bufs=1) as w2pool,
                tc.tile_pool(name="xte", bufs=1) as xtep,
                tc.tile_pool(name="hall", bufs=1) as hallp,
                tc.tile_pool(name="ypool", bufs=1) as ypool,
                tc.tile_pool(name="hps", bufs=2, space="PSUM") as hps,
                tc.tile_pool(name="yps", bufs=1, space="PSUM") as yps,
            ):
                w2_sb = w2pool.tile([P, HC, D], BF16)
                for hc in range(HC):
                    nc.sync.dma_start(w2_sb[:, hc, :], w2_in[:, hc, :])
                h_all = hallp.tile([P, HC, SB], BF16)
                # all dispatch gathers up-front: gpsimd queue is in-order and
                # collective triggers block it, so gathers must precede them
                xTes = []
                for sb in range(NB):
                    xTe = xtep.tile([P, DC, SB], BF16, name=f"xTe{sb}")
                    nc.gpsimd.dma_gather(
                        out_ap=xTe[:],
                        in_ap=xb_all[:],
                        idxs_ap=idx_x[:, sb * (SB // 16):(sb + 1) * (SB // 16)],
                        num_idxs=SB, num_idxs_reg=SB, elem_size=D, transpose=True,
                    )
                    xTes.append(xTe)
                for sb in range(NB):
                    xTe = xTes[sb]
                    for hc in range(HC):
                        h_ps = hps.tile([P, SB], F32, space="PSUM", name="hps")
                        for dc in range(DC):
                            nc.tensor.matmul(
                                h_ps[:],
                                w1_sb[:, hc, dc, :],
                                xTe[:, dc, :],
                                start=(dc == 0), stop=(dc == DC - 1),
                            )
                        nc.scalar.activation(
                            h_all[:, hc, :], h_ps[:], AF.Gelu_apprx_tanh,
                            bias=b1_sb[:, hc:hc + 1],
                        )
                    for half in range(2):
                        y_ts = [
                            [yps.tile([P, 512], F32, space="PSUM", name=f"y{st}{dg}") for dg in range(2)]
                            for st in range(2)
                        ]
                        for hc in range(HC):
                            for st in range(2):
                                so = half * 256 + st * P
                                for dg in range(2):
                                    nc.tensor.matmul(
                                        y_ts[st][dg][:],
                                        h_all[:, hc, so:so + P],
                                        w2_sb[:, hc, dg * 512:(dg + 1) * 512],
                                        start=(hc == 0), stop=False,
                                    )
                        y_sb = ypool.tile([P, 2, D], BF16, name="ysb")
                        for st in range(2):
                            for dg in range(2):
                                nc.tensor.matmul(
                                    y_ts[st][dg][:], ones1[:],
                                    b2_sb[:, dg * 512:(dg + 1) * 512],
                                    start=False, stop=True,
                                )
                                nc.scalar.activation(
                                    y_sb[:, st, dg * 512:(dg + 1) * 512],
                                    y_ts[st][dg][:], AF.Copy,
                                )
                        r0 = sb * SB + half * 256
                        nc.sync.dma_start(
                            yag_in[r0:r0 + 256, :].rearrange("(s p) d -> p s d", p=P),
                            y_sb[:],
                        )
                    nc.gpsimd.collective_compute(
                        "AllGather", OP.bypass, replica_groups=[list(range(E))],
                        ins=[yag_in[sb * SB:(sb + 1) * SB, :].opt()],
                        outs=[yag_out[sb * YCH:(sb + 1) * YCH, :].opt()],
                    )

            # ---------------- P4: home combine ----------------
            with tc.tile_pool(name="homep", bufs=1) as homep:
                g1 = homep.tile([P, NTS, D], BF16)
                nc.gpsimd.dma_gather(
                    out_ap=g1[:], in_ap=yag_out[:], idxs_ap=idx_h1[:],
                    num_idxs=TS, num_idxs_reg=TS, elem_size=D,
                )
                g2 = homep.tile([P, NTS, D], BF16)
                nc.gpsimd.dma_gather(
                    out_ap=g2[:], in_ap=yag_out[:], idxs_ap=idx_h2[:],
                    num_idxs=TS, num_idxs_reg=TS, elem_size=D,
                )
                o1 = homep.tile([P, NTS, D], F32)
                nc.vector.tensor_tensor(
                    o1[:], g1[:],
                    cw1_my[:].rearrange("p c -> p c ()").to_broadcast([P, NTS, D]),
                    op=OP.mult,
                )
                o2 = homep.tile([P, NTS, D], F32)
                nc.vector.tensor_tensor(
                    o2[:], g2[:],
                    cw2_my[:].rearrange("p c -> p c ()").to_broadcast([P, NTS, D]),
                    op=OP.mult,
                )
                nc.vector.tensor_tensor(o1[:], o1[:], o2[:], op=OP.add)
                nc.sync.dma_start(
                    out_sl[:].rearrange("(a p) d -> p a d", p=P), o1[:]
                )

    nc.compile()
    return nc


_NC_CACHE = {}


def _get_nc(debug=False):
    key = f"nc{debug}"
    if key not in _NC_CACHE:
        _NC_CACHE[key] = build_moe(debug)
    return _NC_CACHE[key]


def make_inputs(x, Wg, W1, b1, W2, b2):
    """Host-side sharding: per-core input maps (data-independent prep only)."""
    bf = ml_dtypes.bfloat16
    x = np.ascontiguousarray(np.asarray(x, dtype=np.float32).reshape(T, D))
    wg = np.ascontiguousarray(
        np.asarray(Wg, dtype=np.float32).reshape(DC, P, E).transpose(1, 0, 2)
    )
    p = np.arange(P)
    tm = p // E
    ee = p % E
    msame = (ee[:, None] == ee[None, :]).astype(np.float32)        # [p', p]
    mlow = (msame * (tm[:, None] < tm[None, :])).astype(np.float32)
    erow = np.tile(np.arange(E, dtype=np.float32), (P, 1))
    loall = np.tile(np.arange(P, dtype=np.float32), (P, NT, 1)).astype(bf)
    hiall = np.tile(np.arange(16, dtype=np.float32), (P, NT, 1)).astype(np.float32)
    j = np.arange(NT)
    thi = (j[None, :] * 2 + (p // 64)[:, None]).astype(np.float32)  # (j*128+p)//64
    tlo = (p % 64).astype(np.float32).reshape(P, 1)

    in_maps = []
    for e in range(E):
        w1s = np.ascontiguousarray(
            np.asarray(W1[e], dtype=np.float32)
            .reshape(DC, P, HC, P).transpose(1, 2, 0, 3).astype(bf)
        )
        w2s = np.ascontiguousarray(
            np.asarray(W2[e], dtype=np.float32).reshape(HC, P, D).transpose(1, 0, 2).astype(bf)
        )
        b1s = np.ascontiguousarray(np.asarray(b1[e], dtype=np.float32).reshape(HC, P).T)
        b2r = np.asarray(b2[e], dtype=np.float32).reshape(1, D).astype(bf)
        esel = np.zeros((P, E), dtype=np.float32)
        esel[:, e] = 1.0
        hm64 = np.zeros((P, NT), dtype=np.float32)
        hm64[:, e * NTS:(e + 1) * NTS] = 1.0
        in_maps.append({
            "xs": np.ascontiguousarray(x[e * TS:(e + 1) * TS]),
            "wg": wg, "w1s": w1s, "w2s": w2s, "b1s": b1s, "b2r": b2r,
            "esel": esel, "hm64": hm64, "erow": erow,
            "msame": msame, "mlow": mlow, "loall": loall, "hiall": hiall,
            "thi": thi, "tlo": tlo,
        })
    return in_maps


def kernel(x, Wg, W1, b1, W2, b2):
    nc = _get_nc()
    in_maps = make_inputs(x, Wg, W1, b1, W2, b2)
    res = run_bass_kernel_spmd(nc, in_maps, list(range(E)))
    out = np.concatenate([res.results[e]["out_slice"] for e in range(E)], axis=0)
    return out.reshape(B, S, D).astype(np.float32)


# revision 4
# speedup vs baseline: 2.5961x; 1.1061x over previous
"""MoE (top-2, capacity-dropped) Trainium2 kernel v2 — expert-parallel, 8 cores.

vs baseline:
- Routing logits sharded per core + AllGather (logits 32KB, xb 2.1MB).
- All-expert capacity cumsum in transposed (tm*8+e, block, token) layout;
  cross-tile offsets via two static-mask matmuls (Msame/Mlow).
- slot->token map built ON-CHIP via one-hot factored matmuls (no DRAM
  scatter/gather staging).
- FFN: weights SBUF-resident, N=512 moving operand, h staged in SBUF bf16,
  W2 per 256-slot half (PSUM: 2 h banks + 4 y banks).
- Combine: y slot-major, AllGathered in 4 chunks overlapped with FFN; home
  cores gather their tokens' <=2 expert rows and combine with weights.
"""

import numpy as np
import ml_dtypes

import concourse.bass as bass
import concourse.tile as tile
from concourse import bacc, mybir
from concourse.bass_utils import run_bass_kernel_spmd
from concourse.masks import make_identity

F32 = mybir.dt.float32
BF16 = mybir.dt.bfloat16
I16 = mybir.dt.int16
AF = mybir.ActivationFunctionType
OP = mybir.AluOpType

P = 128
E = 8
B, S, D = 2, 4096, 1024
H = 4096
T = B * S                  # 8192 tokens
C = 2048                   # capacity per expert
NT = T // P                # 64 token tiles
NTS = NT // E              # 8 token tiles per core slice
TS = T // E                # 1024 tokens per core slice
DC = D // P                # 8 d-chunks
HC = H // P                # 32 h-chunks
NB = 4                     # FFN super-blocks (512 slots each)
SB = C // NB               # 512 slots per super-block
YCH = E * SB               # rows per y AllGather chunk (4096)


def _ge_sum(nc, r2, src, levels, name):
    """acc = sum_k [src >= levels[k]] (compare cascade; all f32)."""
    acc = r2.tile(list(src.shape), F32, name=name)
    tmp = r2.tile(list(src.shape), F32, name=name + "t")
    nc.vector.tensor_scalar(acc[:], src[:], float(levels[0]), None, op0=OP.is_ge)
    for lv in levels[1:]:
        nc.vector.tensor_scalar(tmp[:], src[:], float(lv), None, op0=OP.is_ge)
        nc.vector.tensor_tensor(acc[:], acc[:], tmp[:], op=OP.add)
    return acc


def build_moe(debug=False):
    nc = bacc.Bacc("TRN2", target_bir_lowering=False, debug=False, num_devices=E)

    xs_in = nc.dram_tensor("xs", [TS, D], F32, kind="ExternalInput").ap()
    wg_in = nc.dram_tensor("wg", [P, DC, E], F32, kind="ExternalInput").ap()
    w1_in = nc.dram_tensor("w1s", [P, HC, DC, P], BF16, kind="ExternalInput").ap()
    w2_in = nc.dram_tensor("w2s", [P, HC, D], BF16, kind="ExternalInput").ap()
    b1_in = nc.dram_tensor("b1s", [P, HC], F32, kind="ExternalInput").ap()
    b2_in = nc.dram_tensor("b2r", [1, D], BF16, kind="ExternalInput").ap()
    esel_in = nc.dram_tensor("esel", [P, E], F32, kind="ExternalInput").ap()
    hm64_in = nc.dram_tensor("hm64", [P, NT], F32, kind="ExternalInput").ap()
    erow_in = nc.dram_tensor("erow", [P, E], F32, kind="ExternalInput").ap()
    msame_in = nc.dram_tensor("msame", [P, P], F32, kind="ExternalInput").ap()
    mlow_in = nc.dram_tensor("mlow", [P, P], F32, kind="ExternalInput").ap()
    lo_in = nc.dram_tensor("loall", [P, NT, P], BF16, kind="ExternalInput").ap()
    hi_in = nc.dram_tensor("hiall", [P, NT, 16], F32, kind="ExternalInput").ap()
    thi_in = nc.dram_tensor("thi", [P, NT], F32, kind="ExternalInput").ap()
    tlo_in = nc.dram_tensor("tlo", [P, 1], F32, kind="ExternalInput").ap()

    out_sl = nc.dram_tensor("out_slice", [TS, D], F32, kind="ExternalOutput").ap()

    xb_sl_dram = nc.dram_tensor("xb_slice", [TS, D], BF16)
    xb_all = nc.dram_tensor("xb_all", [T, D], BF16, addr_space="Shared")
    lg_sl_dram = nc.dram_tensor("lg_slice", [TS, E], F32)
    lg_all = nc.dram_tensor("lg_all", [T, E], F32, addr_space="Shared")
    yag_in = nc.dram_tensor("yag_in", [C, D], BF16)
    yag_out = nc.dram_tensor("yag_out", [NB * YCH, D], BF16, addr_space="Shared")

    if debug:
        dbg_lg = nc.dram_tensor("dbg_lg", [P, NT, E], F32, kind="ExternalOutput").ap()
        dbg_pos1 = nc.dram_tensor("dbg_pos1", [P, NT], F32, kind="ExternalOutput").ap()
        dbg_pos2 = nc.dram_tensor("dbg_pos2", [P, NT], F32, kind="ExternalOutput").ap()
        dbg_map = nc.dram_tensor("dbg_map", [P, 16], F32, kind="ExternalOutput").ap()
        dbg_gs1 = nc.dram_tensor("dbg_gs1", [P, NT], F32, kind="ExternalOutput").ap()
        dbg_gs2 = nc.dram_tensor("dbg_gs2", [P, NT], F32, kind="ExternalOutput").ap()
        dbg_cw1 = nc.dram_tensor("dbg_cw1", [P, NT], F32, kind="ExternalOutput").ap()
        dbg_cw2 = nc.dram_tensor("dbg_cw2", [P, NT], F32, kind="ExternalOutput").ap()

    with tile.TileContext(nc) as tc:
        with (
            tc.tile_pool(name="const", bufs=1) as const,
            tc.tile_pool(name="persist", bufs=1) as persist,
            tc.tile_pool(name="w1pool", bufs=1) as w1pool,
        ):
            # ---------------- constants ----------------
            ident = const.tile([P, P], F32)
            make_identity(nc, ident[:])
            wg_sb = const.tile([P, DC, E], F32)
            nc.sync.dma_start(wg_sb[:], wg_in[:])
            b1_sb = const.tile([P, HC], F32)
            nc.sync.dma_start(b1_sb[:], b1_in[:])
            b2_sb = const.tile([1, D], BF16)
            nc.sync.dma_start(b2_sb[:], b2_in[:])
            ones1 = const.tile([1, P], BF16)
            nc.vector.memset(ones1[:], 1.0)

            # W1 resident (hc-major layout) — scalar-queue DMA so the x-slice
            # loads on the sync queue aren't starved behind 8MB of weights
            w1_sb = w1pool.tile([P, HC, DC, P], BF16)
            for hc in range(HC):
                nc.scalar.dma_start(w1_sb[:, hc, :, :], w1_in[:, hc, :, :])

            # ---------------- P1: slice logits + bf16 cast ----------------
            with (
                tc.tile_pool(name="p1x", bufs=1) as p1x,
                tc.tile_pool(name="p1xb", bufs=3) as p1xb,
                tc.tile_pool(name="p1xt", bufs=3) as p1xt,
                tc.tile_pool(name="p1lg", bufs=2) as p1lg,
                tc.tile_pool(name="p1pst", bufs=2, space="PSUM") as p1pst,
                tc.tile_pool(name="p1psl", bufs=2, space="PSUM") as p1psl,
            ):
                # pass A: logits only (critical path to the lg AllGather)
                x_tiles = []
                for i in range(NTS):
                    x_sb = p1x.tile([P, D], F32, name=f"x{i}")
                    nc.sync.dma_start(x_sb[:], xs_in[i * P:(i + 1) * P, :])
                    x_tiles.append(x_sb)
                    lg_ps = p1psl.tile([P, E], F32, space="PSUM")
                    for half in range(2):
                        tr_ps = p1pst.tile([P, 4 * P], F32, space="PSUM")
                        for j in range(4):
                            dc = half * 4 + j
                            nc.tensor.matmul(
                                tr_ps[:, j * P:(j + 1) * P],
                                x_sb[:, dc * P:(dc + 1) * P],
                                ident[:],
                                is_transpose=True,
                                start=(j == 0),
                                stop=(j == 3),
                            )
                        xt_sb = p1xt.tile([P, 4 * P], F32)
                        nc.vector.tensor_copy(xt_sb[:], tr_ps[:])
                        for j in range(4):
                            dc = half * 4 + j
                            nc.tensor.matmul(
                                lg_ps[:],
                                xt_sb[:, j * P:(j + 1) * P],
                                wg_sb[:, dc, :],
                                start=(dc == 0),
                                stop=(dc == DC - 1),
                            )
                    lg_sb = p1lg.tile([P, E], F32)
                    nc.vector.tensor_copy(lg_sb[:], lg_ps[:])
                    nc.sync.dma_start(lg_sl_dram[i * P:(i + 1) * P, :], lg_sb[:])

                nc.gpsimd.collective_compute(
                    "AllGather", OP.bypass, replica_groups=[list(range(E))],
                    ins=[lg_sl_dram[:].opt()], outs=[lg_all[:].opt()],
                )

                # pass B: bf16 cast + staging (xb_all only needed at dispatch)
                for i in range(NTS):
                    xb_sb = p1xb.tile([P, D], BF16)
                    nc.vector.tensor_copy(xb_sb[:], x_tiles[i][:])
                    nc.sync.dma_start(xb_sl_dram[i * P:(i + 1) * P, :], xb_sb[:])

            nc.gpsimd.collective_compute(
                "AllGather", OP.bypass, replica_groups=[list(range(E))],
                ins=[xb_sl_dram[:].opt()], outs=[xb_all[:].opt()],
            )

            # persist tiles used across phases
            cw1_my = persist.tile([P, E], F32)
            cw2_my = persist.tile([P, E], F32)
            idx_h1 = persist.tile([P, TS // 16], I16)   # home gather idx (wrapped)
            idx_h2 = persist.tile([P, TS // 16], I16)
            idx_x = persist.tile([P, C // 16], I16)     # dispatch gather idx

            # ---------------- P2: routing (replicated, from lg_all) ------
            with (
                tc.tile_pool(name="r2", bufs=1) as r2,
                tc.tile_pool(name="ohps", bufs=2, space="PSUM") as ohps,
                tc.tile_pool(name="mmps", bufs=1, space="PSUM") as mmps,
                tc.tile_pool(name="bkps", bufs=1, space="PSUM") as bkps,
                tc.tile_pool(name="mapps", bufs=1, space="PSUM") as mapps,
            ):
                esel_sb = r2.tile([P, E], F32)
                nc.sync.dma_start(esel_sb[:], esel_in[:])
                hm64_sb = r2.tile([P, NT], F32)
                nc.sync.dma_start(hm64_sb[:], hm64_in[:])
                erow_sb = r2.tile([P, E], F32)
                nc.sync.dma_start(erow_sb[:], erow_in[:])
                msame_sb = r2.tile([P, P], F32)
                nc.sync.dma_start(msame_sb[:], msame_in[:])
                mlow_sb = r2.tile([P, P], F32)
                nc.sync.dma_start(mlow_sb[:], mlow_in[:])
                lo_sb = r2.tile([P, NT, P], BF16)
                nc.sync.dma_start(lo_sb[:], lo_in[:])
                hi_sb = r2.tile([P, NT, 16], F32)
                nc.sync.dma_start(hi_sb[:], hi_in[:])
                thi_sb = r2.tile([P, NT], F32)
                nc.sync.dma_start(thi_sb[:], thi_in[:])
                tlo_sb = r2.tile([P, 1], F32)
                nc.sync.dma_start(tlo_sb[:], tlo_in[:])

                lgt = r2.tile([P, NT, E], F32)
                nc.sync.dma_start(
                    lgt[:], lg_all[:].rearrange("(a p) e -> p a e", p=P)
                )
                if debug:
                    nc.sync.dma_start(dbg_lg[:], lgt[:])
                # ---- top-2 (token-major) ----
                m1 = r2.tile([P, NT], F32)
                nc.vector.tensor_reduce(m1[:], lgt[:], axis=mybir.AxisListType.X, op=OP.max)
                oh1 = r2.tile([P, NT, E], F32)
                nc.vector.tensor_tensor(
                    oh1[:], lgt[:], m1[:].rearrange("p t -> p t ()").to_broadcast([P, NT, E]),
                    op=OP.is_equal,
                )
                masked = r2.tile([P, NT, E], F32)
                nc.vector.tensor_scalar(masked[:], oh1[:], -1e9, None, op0=OP.mult)
                nc.vector.tensor_tensor(masked[:], masked[:], lgt[:], op=OP.add)
                m2 = r2.tile([P, NT], F32)
                nc.vector.tensor_reduce(m2[:], masked[:], axis=mybir.AxisListType.X, op=OP.max)
                oh2 = r2.tile([P, NT, E], F32)
                nc.vector.tensor_tensor(
                    oh2[:], masked[:], m2[:].rearrange("p t -> p t ()").to_broadcast([P, NT, E]),
                    op=OP.is_equal,
                )
                delta = r2.tile([P, NT], F32)
                nc.vector.tensor_tensor(delta[:], m2[:], m1[:], op=OP.subtract)
                wr1 = r2.tile([P, NT], F32)
                nc.scalar.activation(wr1[:], delta[:], AF.Sigmoid, scale=-1.0)
                wr2 = r2.tile([P, NT], F32)
                nc.scalar.activation(wr2[:], delta[:], AF.Sigmoid)

                # ---- all-expert capacity cumsum, P2 layout (tm*8+e, b, tok) ----
                ohs = [oh1, oh2]
                csm1T = []      # token-major (cs-1)*keep*oh per rank  [P, NT, E]
                kT = []         # token-major keep*oh per rank         [P, NT, E]
                base1 = None
                for r in range(2):
                    ohT_ps = ohps.tile([P, 4 * P], F32, space="PSUM", name="ohT")
                    ohsv = ohs[r][:].rearrange("p a e -> p (a e)")
                    for b in range(4):
                        nc.tensor.matmul(
                            ohT_ps[:, b * P:(b + 1) * P],
                            ohsv[:, b * P:(b + 1) * P],
                            ident[:],
                            is_transpose=True,
                            start=(b == 0), stop=(b == 3),
                        )
                    ohT = r2.tile([P, 4, P], F32, name=f"ohTs{r}")
                    nc.vector.tensor_copy(ohT[:], ohT_ps[:].rearrange("p (b t) -> p b t", b=4))
                    ic = r2.tile([P, 4, P], F32, name=f"ic{r}")
                    for b in range(4):
                        nc.vector.tensor_tensor_scan(
                            ic[:, b, :], ohT[:, b, :], ohT[:, b, :], 0.0,
                            op0=OP.add, op1=OP.bypass,
                        )
                    cnt = r2.tile([P, 4], F32, name=f"cnt{r}")
                    nc.vector.tensor_copy(cnt[:], ic[:, :, P - 1])
                    # cross-tile offsets: same-expert block totals + intra lower
                    mm_ps = mmps.tile([P, 8], F32, space="PSUM", name="mm")
                    nc.tensor.matmul(mm_ps[:, 0:4], msame_sb[:], cnt[:], start=True, stop=False)
                    nc.tensor.matmul(mm_ps[:, 4:8], mlow_sb[:], cnt[:], start=False, stop=True)
                    mm_sb = r2.tile([P, 8], F32, name=f"mmsb{r}")
                    nc.vector.tensor_copy(mm_sb[:], mm_ps[:])
                    btot_i = r2.tile([P, 4], F32, name=f"bti{r}")
                    nc.vector.tensor_tensor_scan(
                        btot_i[:], mm_sb[:, 0:4], mm_sb[:, 0:4], 0.0,
                        op0=OP.add, op1=OP.bypass,
                    )
                    offs = r2.tile([P, 4], F32, name=f"offs{r}")
                    nc.vector.tensor_tensor(offs[:], btot_i[:], mm_sb[:, 0:4], op=OP.subtract)
                    nc.vector.tensor_tensor(offs[:], offs[:], mm_sb[:, 4:8], op=OP.add)
                    if r == 1:
                        nc.vector.tensor_scalar(offs[:], offs[:], base1[:], None, op0=OP.add)
                    cs = r2.tile([P, 4, P], F32, name=f"cs{r}")
                    for b in range(4):
                        nc.vector.tensor_scalar(
                            cs[:, b, :], ic[:, b, :], offs[:, b:b + 1], None, op0=OP.add
                        )
                    if r == 0:
                        # rank-1 base: min(total rank-0 assigned per expert, C);
                        # mm_sb[:, 0:4] holds per-expert block totals (Msame).
                        n0 = r2.tile([P, 1], F32)
                        nc.vector.tensor_reduce(n0[:], mm_sb[:, 0:4], axis=mybir.AxisListType.X, op=OP.add)
                        base1 = r2.tile([P, 1], F32)
                        nc.vector.tensor_scalar(base1[:], n0[:], float(C), None, op0=OP.min)
                    keep = r2.tile([P, 4, P], F32, name=f"keep{r}")
                    nc.vector.tensor_scalar(keep[:], cs[:], float(C), None, op0=OP.is_le)
                    kk = r2.tile([P, 4, P], F32, name=f"kk{r}")
                    nc.vector.tensor_tensor(kk[:], keep[:], ohT[:], op=OP.mult)
                    ksl = r2.tile([P, 4, P], F32, name=f"ksl{r}")
                    nc.vector.tensor_scalar(ksl[:], cs[:], -1.0, None, op0=OP.add)
                    nc.vector.tensor_tensor(ksl[:], ksl[:], kk[:], op=OP.mult)
                    # transpose back to token-major (two 1-bank psum tiles)
                    bk1 = bkps.tile([P, 4 * P], F32, space="PSUM", name="bk1")
                    bk2 = bkps.tile([P, 4 * P], F32, space="PSUM", name="bk2")
                    for b in range(4):
                        nc.tensor.matmul(
                            bk1[:, b * P:(b + 1) * P], ksl[:, b, :], ident[:],
                            is_transpose=True, start=(b == 0), stop=(b == 3),
                        )
                    for b in range(4):
                        nc.tensor.matmul(
                            bk2[:, b * P:(b + 1) * P], kk[:, b, :], ident[:],
                            is_transpose=True, start=(b == 0), stop=(b == 3),
                        )
                    cT = r2.tile([P, NT, E], F32, name=f"cT{r}")
                    nc.vector.tensor_copy(cT[:], bk1[:].rearrange("p (a e) -> p a e", e=E))
                    kTr = r2.tile([P, NT, E], F32, name=f"kTr{r}")
                    nc.vector.tensor_copy(kTr[:], bk2[:].rearrange("p (a e) -> p a e", e=E))
                    csm1T.append(cT)
                    kT.append(kTr)

                # ---- home-side indices (token-major, all tiles) ----
                esel_b = esel_sb[:].rearrange("p e -> p () e").to_broadcast([P, NT, E])
                erow_b = erow_sb[:].rearrange("p e -> p () e").to_broadcast([P, NT, E])
                tmp3 = r2.tile([P, NT, E], F32)
                gs = []
                cwf = []
                for r in range(2):
                    pos = r2.tile([P, NT], F32, name=f"pos{r}")
                    nc.vector.tensor_reduce(pos[:], csm1T[r][:], axis=mybir.AxisListType.X, op=OP.add)
                    keep_s = r2.tile([P, NT], F32, name=f"ks{r}")
                    nc.vector.tensor_reduce(keep_s[:], kT[r][:], axis=mybir.AxisListType.X, op=OP.max)
                    nc.vector.tensor_tensor(tmp3[:], ohs[r][:], erow_b, op=OP.mult)
                    es = r2.tile([P, NT], F32, name=f"es{r}")
                    nc.vector.tensor_reduce(es[:], tmp3[:], axis=mybir.AxisListType.X, op=OP.max)
                    # chunked-AG row index: 2048*(s//256) + 256*e + s%256
                    q = _ge_sum(nc, r2, pos, [256.0 * k for k in range(1, 8)], f"q{r}")
                    g = r2.tile([P, NT], F32, name=f"g{r}")
                    rem = r2.tile([P, NT], F32, name=f"rm{r}")
                    nc.vector.tensor_scalar(rem[:], q[:], -256.0, None, op0=OP.mult)
                    nc.vector.tensor_tensor(rem[:], rem[:], pos[:], op=OP.add)
                    nc.vector.tensor_scalar(g[:], q[:], 2048.0, None, op0=OP.mult)
                    t2 = r2.tile([P, NT], F32, name=f"t2{r}")
                    nc.vector.tensor_scalar(t2[:], es[:], 256.0, None, op0=OP.mult)
                    nc.vector.tensor_tensor(g[:], g[:], t2[:], op=OP.add)
                    nc.vector.tensor_tensor(g[:], g[:], rem[:], op=OP.add)
                    cw = r2.tile([P, NT], F32, name=f"cw{r}")
                    wsrc = wr1 if r == 0 else wr2
                    nc.vector.tensor_tensor(cw[:], wsrc[:], keep_s[:], op=OP.mult)
                    gs.append(g)
                    cwf.append(cw)
                    if debug:
                        nc.sync.dma_start([dbg_gs1, dbg_gs2][r][:], g[:])
                        nc.sync.dma_start([dbg_cw1, dbg_cw2][r][:], cw[:])
                        nc.sync.dma_start([dbg_pos1, dbg_pos2][r][:], pos[:])

                # select MY home block (hm64 mask + log-fold), build wrapped idx
                for r in range(2):
                    msk_g = r2.tile([P, NT], F32, name=f"mg{r}")
                    nc.vector.tensor_tensor(msk_g[:], gs[r][:], hm64_sb[:], op=OP.mult)
                    msk_c = r2.tile([P, NT], F32, name=f"mc{r}")
                    nc.vector.tensor_tensor(msk_c[:], cwf[r][:], hm64_sb[:], op=OP.mult)
                    for half in (32, 16, 8):
                        nc.vector.tensor_tensor(
                            msk_g[:, 0:half], msk_g[:, 0:half], msk_g[:, half:2 * half], op=OP.add
                        )
                        nc.vector.tensor_tensor(
                            msk_c[:, 0:half], msk_c[:, 0:half], msk_c[:, half:2 * half], op=OP.add
                        )
                    nc.vector.tensor_copy([cw1_my, cw2_my][r][:], msk_c[:, 0:E])
                    gi = r2.tile([P, E], I16, name=f"gi{r}")
                    nc.vector.tensor_copy(gi[:], msk_g[:, 0:E])
                    sh16 = [(i + 16) % 32 for i in range(32)]
                    gish = r2.tile([P, E], I16, name=f"gish{r}")
                    nc.vector.stream_shuffle(gish[:], gi[:], sh16)
                    idxh = [idx_h1, idx_h2][r]
                    idxh_v = idxh[0:16, :].rearrange("r (c q) -> r c q", q=8)
                    for qq in range(8):
                        src = gi if qq % 2 == 0 else gish
                        nc.vector.tensor_copy(
                            idxh_v[:, :, qq],
                            src[(qq // 2) * 32:(qq // 2) * 32 + 16, :],
                        )
                    for k in range(1, 8):
                        nc.sync.dma_start(idxh[16 * k:16 * (k + 1), :], idxh[0:16, :])

                # ---- expert-side slot->token map (my expert) ----
                ksl_e = r2.tile([P, NT], F32)
                k_e = r2.tile([P, NT], F32)
                acc = r2.tile([P, NT], F32)
                for r in range(2):
                    nc.vector.tensor_tensor(tmp3[:], csm1T[r][:], esel_b, op=OP.mult)
                    nc.vector.tensor_reduce(
                        (acc if r else ksl_e)[:], tmp3[:], axis=mybir.AxisListType.X, op=OP.add
                    )
                    if r:
                        nc.vector.tensor_tensor(ksl_e[:], ksl_e[:], acc[:], op=OP.add)
                    nc.vector.tensor_tensor(tmp3[:], kT[r][:], esel_b, op=OP.mult)
                    nc.vector.tensor_reduce(
                        (acc if r else k_e)[:], tmp3[:], axis=mybir.AxisListType.X, op=OP.max
                    )
                    if r:
                        nc.vector.tensor_tensor(k_e[:], k_e[:], acc[:], op=OP.max)
                # chi = s//128 in [0,16), remc = s%128 via two-level cascade
                q8 = _ge_sum(nc, r2, ksl_e, [512.0, 1024.0, 1536.0], "q8")
                s1 = r2.tile([P, NT], F32)
                nc.vector.tensor_scalar(s1[:], q8[:], -512.0, None, op0=OP.mult)
                nc.vector.tensor_tensor(s1[:], s1[:], ksl_e[:], op=OP.add)
                c3 = _ge_sum(nc, r2, s1, [128.0, 256.0, 384.0], "c3")
                chi = r2.tile([P, NT], F32)
                nc.vector.tensor_scalar(chi[:], q8[:], 4.0, None, op0=OP.mult)
                nc.vector.tensor_tensor(chi[:], chi[:], c3[:], op=OP.add)
                remc = r2.tile([P, NT], F32)
                nc.vector.tensor_scalar(remc[:], c3[:], -128.0, None, op0=OP.mult)
                nc.vector.tensor_tensor(remc[:], remc[:], s1[:], op=OP.add)
                ktlo = r2.tile([P, NT], F32)
                nc.vector.tensor_scalar(ktlo[:], k_e[:], tlo_sb[:], None, op0=OP.mult)
                kthi = r2.tile([P, NT], F32)
                nc.vector.tensor_tensor(kthi[:], k_e[:], thi_sb[:], op=OP.mult)

                o_all = r2.tile([P, NT, P], BF16)   # [s%128 == lo]
                nc.vector.tensor_tensor(
                    o_all[:], lo_sb[:],
                    remc[:].rearrange("p t -> p t ()").to_broadcast([P, NT, P]),
                    op=OP.is_equal,
                )
                v0 = r2.tile([P, NT, 16], F32)
                nc.vector.tensor_tensor(
                    v0[:], hi_sb[:],
                    chi[:].rearrange("p t -> p t ()").to_broadcast([P, NT, 16]),
                    op=OP.is_equal,
                )
                v_all = r2.tile([P, NT, 2, 16], BF16)
                nc.vector.tensor_tensor(
                    v_all[:, :, 0, :], v0[:],
                    ktlo[:].rearrange("p t -> p t ()").to_broadcast([P, NT, 16]),
                    op=OP.mult,
                )
                nc.vector.tensor_tensor(
                    v_all[:, :, 1, :], v0[:],
                    kthi[:].rearrange("p t -> p t ()").to_broadcast([P, NT, 16]),
                    op=OP.mult,
                )
                map_ps = mapps.tile([P, 2, 16], F32, space="PSUM")
                for j in range(NT):
                    nc.tensor.matmul(
                        map_ps[:].rearrange("p a b -> p (a b)"),
                        o_all[:, j, :],
                        v_all[:, j, :, :].rearrange("p a b -> p (a b)"),
                        start=(j == 0), stop=(j == NT - 1),
                    )
                map_sb = r2.tile([P, 2, 16], F32)
                nc.vector.tensor_copy(map_sb[:], map_ps[:])
                map_tok = r2.tile([P, 16], F32)
                nc.vector.tensor_scalar(map_tok[:], map_sb[:, 1, :], 64.0, None, op0=OP.mult)
                nc.vector.tensor_tensor(map_tok[:], map_tok[:], map_sb[:, 0, :], op=OP.add)
                if debug:
                    nc.sync.dma_start(dbg_map[:], map_tok[:])
                mi = r2.tile([P, 16], I16)
                nc.vector.tensor_copy(mi[:], map_tok[:])
                sh16 = [(i + 16) % 32 for i in range(32)]
                mish = r2.tile([P, 16], I16)
                nc.vector.stream_shuffle(mish[:], mi[:], sh16)
                idxx_v = idx_x[0:16, :].rearrange("r (h q) -> r h q", q=8)
                for qq in range(8):
                    src = mi if qq % 2 == 0 else mish
                    nc.vector.tensor_copy(
                        idxx_v[:, :, qq],
                        src[(qq // 2) * 32:(qq // 2) * 32 + 16, :],
                    )
                for k in range(1, 8):
                    nc.sync.dma_start(idx_x[16 * k:16 * (k + 1), :], idx_x[0:16, :])

            # ---------------- P3: FFN ----------------
            with (
                tc.tile_pool(name="w2pool", bufs=1) as w2pool,
                tc.tile_pool(name="xte", bufs=1) as xtep,
                tc.tile_pool(name="hall", bufs=1) as hallp,
                tc.tile_pool(name="ypool", bufs=1) as ypool,
                tc.tile_pool(name="hps", bufs=2, space="PSUM") as hps,
                tc.tile_pool(name="yps", bufs=1, space="PSUM") as yps,
            ):
                w2_sb = w2pool.tile([P, HC, D], BF16)
                for hc in range(HC):
                    nc.sync.dma_start(w2_sb[:, hc, :], w2_in[:, hc, :])
                h_all = hallp.tile([P, HC, SB], BF16)
                # all dispatch gathers up-front: gpsimd queue is in-order and
                # collective triggers block it, so gathers must precede them
                xTes = []
                for sb in range(NB):
                    xTe = xtep.tile([P, DC, SB], BF16, name=f"xTe{sb}")
                    nc.gpsimd.dma_gather(
                        out_ap=xTe[:],
                        in_ap=xb_all[:],
                        idxs_ap=idx_x[:, sb * (SB // 16):(sb + 1) * (SB // 16)],
                        num_idxs=SB, num_idxs_reg=SB, elem_size=D, transpose=True,
                    )
                    xTes.append(xTe)
                for sb in range(NB):
                    xTe = xTes[sb]
                    for hc in range(HC):
                        h_ps = hps.tile([P, SB], F32, space="PSUM", name="hps")
                        for dc in range(DC):
                            nc.tensor.matmul(
                                h_ps[:],
                                w1_sb[:, hc, dc, :],
                                xTe[:, dc, :],
                                start=(dc == 0), stop=(dc == DC - 1),
                            )
                        nc.scalar.activation(
                            h_all[:, hc, :], h_ps[:], AF.Gelu_apprx_tanh,
                            bias=b1_sb[:, hc:hc + 1],
                        )
                    for half in range(2):
                        y_ts = [
                            [yps.tile([P, 512], F32, space="PSUM", name=f"y{st}{dg}") for dg in range(2)]
                            for st in range(2)
                        ]
                        for hc in range(HC):
                            for st in range(2):
                                so = half * 256 + st * P
                                for dg in range(2):
                                    nc.tensor.matmul(
                                        y_ts[st][dg][:],
                                        h_all[:, hc, so:so + P],
                                        w2_sb[:, hc, dg * 512:(dg + 1) * 512],
                                        start=(hc == 0), stop=False,
                                    )
                        y_sb = ypool.tile([P, 2, D], BF16, name="ysb")
                        for st in range(2):
                            for dg in range(2):
                                nc.tensor.matmul(
                                    y_ts[st][dg][:], ones1[:],
                                    b2_sb[:, dg * 512:(dg + 1) * 512],
                                    start=False, stop=True,
                                )
                                nc.scalar.activation(
                                    y_sb[:, st, dg * 512:(dg + 1) * 512],
                                    y_ts[st][dg][:], AF.Copy,
                                )
                        r0 = sb * SB + half * 256
                        nc.sync.dma_start(
                            yag_in[r0:r0 + 256, :].rearrange("(s p) d -> p s d", p=P),
                            y_sb[:],
                        )
                        ch = 2 * sb + half
                        nc.gpsimd.collective_compute(
                            "AllGather", OP.bypass, replica_groups=[list(range(E))],
                            ins=[yag_in[ch * 256:(ch + 1) * 256, :].opt()],
                            outs=[yag_out[ch * 2048:(ch + 1) * 2048, :].opt()],
                        )

            # ---------------- P4: home combine (2 pipelined halves) -------
            NH = NTS // 2
            with tc.tile_pool(name="homep", bufs=2) as homep:
                for hh in range(2):
                    c0 = hh * NH
                    g1 = homep.tile([P, NH, D], BF16, name="g1")
                    nc.gpsimd.dma_gather(
                        out_ap=g1[:], in_ap=yag_out[:],
                        idxs_ap=idx_h1[:, c0 * 8:(c0 + NH) * 8],
                        num_idxs=NH * P, num_idxs_reg=NH * P, elem_size=D,
                    )
                    g2 = homep.tile([P, NH, D], BF16, name="g2")
                    nc.gpsimd.dma_gather(
                        out_ap=g2[:], in_ap=yag_out[:],
                        idxs_ap=idx_h2[:, c0 * 8:(c0 + NH) * 8],
                        num_idxs=NH * P, num_idxs_reg=NH * P, elem_size=D,
                    )
                    o1 = homep.tile([P, NH, D], F32, name="o1")
                    nc.vector.tensor_tensor(
                        o1[:], g1[:],
                        cw1_my[:, c0:c0 + NH].rearrange("p c -> p c ()").to_broadcast([P, NH, D]),
                        op=OP.mult,
                    )
                    o2 = homep.tile([P, NH, D], F32, name="o2")
                    nc.vector.tensor_tensor(
                        o2[:], g2[:],
                        cw2_my[:, c0:c0 + NH].rearrange("p c -> p c ()").to_broadcast([P, NH, D]),
                        op=OP.mult,
                    )
                    nc.vector.tensor_tensor(o1[:], o1[:], o2[:], op=OP.add)
                    nc.sync.dma_start(
                        out_sl[c0 * P:(c0 + NH) * P, :].rearrange("(a p) d -> p a d", p=P),
                        o1[:],
                    )

    nc.compile()
    return nc


_NC_CACHE = {}


def _get_nc(debug=False):
    key = f"nc{debug}"
    if key not in _NC_CACHE:
        _NC_CACHE[key] = build_moe(debug)
    return _NC_CACHE[key]


def make_inputs(x, Wg, W1, b1, W2, b2):
    """Host-side sharding: per-core input maps (data-independent prep only)."""
    bf = ml_dtypes.bfloat16
    x = np.ascontiguousarray(np.asarray(x, dtype=np.float32).reshape(T, D))
    wg = np.ascontiguousarray(
        np.asarray(Wg, dtype=np.float32).reshape(DC, P, E).transpose(1, 0, 2)
    )
    p = np.arange(P)
    tm = p // E
    ee = p % E
    msame = (ee[:, None] == ee[None, :]).astype(np.float32)        # [p', p]
    mlow = (msame * (tm[:, None] < tm[None, :])).astype(np.float32)
    erow = np.tile(np.arange(E, dtype=np.float32), (P, 1))
    loall = np.tile(np.arange(P, dtype=np.float32), (P, NT, 1)).astype(bf)
    hiall = np.tile(np.arange(16, dtype=np.float32), (P, NT, 1)).astype(np.float32)
    j = np.arange(NT)
    thi = (j[None, :] * 2 + (p // 64)[:, None]).astype(np.float32)  # (j*128+p)//64
    tlo = (p % 64).astype(np.float32).reshape(P, 1)

    in_maps = []
    for e in range(E):
        w1s = np.ascontiguousarray(
            np.asarray(W1[e], dtype=np.float32)
            .reshape(DC, P, HC, P).transpose(1, 2, 0, 3).astype(bf)
        )
        w2s = np.ascontiguousarray(
            np.asarray(W2[e], dtype=np.float32).reshape(HC, P, D).transpose(1, 0, 2).astype(bf)
        )
        b1s = np.ascontiguousarray(np.asarray(b1[e], dtype=np.float32).reshape(HC, P).T)
        b2r = np.asarray(b2[e], dtype=np.float32).reshape(1, D).astype(bf)
        esel = np.zeros((P, E), dtype=np.float32)
        esel[:, e] = 1.0
        hm64 = np.zeros((P, NT), dtype=np.float32)
        hm64[:, e * NTS:(e + 1) * NTS] = 1.0
        in_maps.append({
            "xs": np.ascontiguousarray(x[e * TS:(e + 1) * TS]),
            "wg": wg, "w1s": w1s, "w2s": w2s, "b1s": b1s, "b2r": b2r,
            "esel": esel, "hm64": hm64, "erow": erow,
            "msame": msame, "mlow": mlow, "loall": loall, "hiall": hiall,
            "thi": thi, "tlo": tlo,
        })
    return in_maps


def kernel(x, Wg, W1, b1, W2, b2):
    nc = _get_nc()
    in_maps = make_inputs(x, Wg, W1, b1, W2, b2)
    res = run_bass_kernel_spmd(nc, in_maps, list(range(E)))
    out = np.concatenate([res.results[e]["out_slice"] for e in range(E)], axis=0)
    return out.reshape(B, S, D).astype(np.float32)
